# revision 5
# baseline (speedup 1.0000x reference)
"""Trainium2 Bass kernel for nn_BuildCostVolume (stereo cost volume + Mamba scan).

Sharding: disparity axis (24) split as 3 per core across 8 cores.

Per-core algorithm (core k handles disparities d = 3k+j, j in 0..2):
  - Host pre-shifts featuresR right by 3k (zero-filled); in-kernel access
    patterns add the per-j shift (j in {0,1,2} is compile-time, SPMD-safe).
  - Projections u/dt/B/C/Dterm are computed from L and shifted-R features with
    even/odd split weights (channel interleave trick), on PE in float32r.
  - dt = softplus via Exp + Ln(x+1) on ACT (no softplus table on trn2).
  - Decay a = exp(A[e,s] * dt) via ACT per-partition scale, in an
    (s-pair x e) = 128-partition layout (e-duplicated weights make the
    duplication free at projection time).
  - Mamba recurrence h = a*h + b via DVE tensor_tensor_scan over flattened
    (row, w) with a[w=0]=0 so each image row restarts the scan.
  - y/cost contraction via PE: block-diag W_out fold, C multiply at PSUM
    eviction, partition-sum + precomputed D-term matmul.
  - Channel attention (masked avg/max pool + MLP) and spatial attention
    computed on small repacked layouts; output written [j,h,g,w] and
    transposed on host.
"""
import os
import numpy as np

C, H, W, DV = 32, 64, 128, 24
_NCH_ENV = int(os.environ.get("KERNEL_NCH", "6"))
_SKIP_EPI = bool(int(os.environ.get("KERNEL_SKIP_EPI", "0")))
_SKIP_PAIRS = bool(int(os.environ.get("KERNEL_SKIP_PAIRS", "0")))
_STREAMS = os.environ.get("KERNEL_STREAMS", "dub")
_ITERS = int(os.environ.get("KERNEL_ITERS", "1"))
E, S, G = 64, 8, 8
NCORES, JD = 8, 3          # cores, disparities per core
PAD = 8                    # leading zero columns in feature tensors
HH = 32                    # h rows per chunk
NCH = 6                    # chunks = (j, h-half)
CCOLS = HH * W             # 4096 columns per chunk
HW = H * W                 # 8192

_compiled = {}


def _f32(x):
    return np.ascontiguousarray(np.asarray(x, np.float32))


def _build_program():
    import concourse.bacc as bacc
    import concourse.mybir as mybir
    from concourse.tile import TileContext

    F32 = mybir.dt.float32
    F32R = mybir.dt.float32r
    BF16 = mybir.dt.bfloat16
    AF = mybir.ActivationFunctionType
    AX = mybir.AxisListType
    OP = mybir.AluOpType

    nc = bacc.Bacc("TRN2", target_bir_lowering=False, debug=False,
                   num_devices=NCORES)

    feat_d = nc.dram_tensor("feat", [C, 2 * (PAD + HW)], F32R, kind="ExternalInput").ap()
    wse_d = nc.dram_tensor("wse", [2 * C, 576], F32R, kind="ExternalInput").ap()
    wbf_d = nc.dram_tensor("wbf", [128, 48], BF16, kind="ExternalInput").ap()
    avec_d = nc.dram_tensor("avec", [128, 8], F32, kind="ExternalInput").ap()
    umask_d = nc.dram_tensor("umask", [32, JD * W], BF16, kind="ExternalInput").ap()
    mneg_d = nc.dram_tensor("mneg", [G, JD * W], BF16, kind="ExternalInput").ap()
    invc_d = nc.dram_tensor("invc", [G, JD], F32, kind="ExternalInput").ap()
    wsp_d = nc.dram_tensor("wsp", [128, 4], F32, kind="ExternalInput").ap()
    mlp_d = nc.dram_tensor("mlp", [G, 24], F32, kind="ExternalInput").ap()
    out_d = nc.dram_tensor("out", [JD * H, G, W], F32, kind="ExternalOutput").ap()

    with TileContext(nc) as tc:
        with tc.tile_pool(name="const", bufs=1) as cpool, \
             tc.tile_pool(name="ftp", bufs=2) as ftp, \
             tc.tile_pool(name="dtmp", bufs=1) as dtmpp, \
             tc.tile_pool(name="dt2", bufs=2) as dt2p, \
             tc.tile_pool(name="dtu2", bufs=2) as dtu2p, \
             tc.tile_pool(name="bc", bufs=2) as bcp, \
             tc.tile_pool(name="bb", bufs=2) as bbp, \
             tc.tile_pool(name="csm", bufs=1) as csmp, \
             tc.tile_pool(name="apool", bufs=2) as apl, \
             tc.tile_pool(name="bpool", bufs=2) as bpl, \
             tc.tile_pool(name="hpool", bufs=4) as hpl, \
             tc.tile_pool(name="tpool", bufs=2) as tpl, \
             tc.tile_pool(name="cstg", bufs=1) as cstgp, \
             tc.tile_pool(name="epi", bufs=1) as epi, \
             tc.tile_pool(name="pproj", bufs=2, space="PSUM") as pproj, \
             tc.tile_pool(name="pz", bufs=1, space="PSUM") as pz, \
             tc.tile_pool(name="pc", bufs=1, space="PSUM") as pc:

            _ld = mybir.InstLoadActFuncSet(
                name=nc.get_next_instruction_name(), act_func_set_id=6,
                ins=[], outs=[])
            nc.scalar.add_instruction(_ld)
            wseL = cpool.tile([C, 576], F32R)
            nc.sync.dma_start(wseL[:], wse_d[0:C, :])
            wseR = cpool.tile([C, 576], F32R)
            nc.sync.dma_start(wseR[:], wse_d[C:2 * C, :])
            wbf = cpool.tile([128, 48], BF16)
            nc.sync.dma_start(wbf[:], wbf_d[:])
            avec = cpool.tile([128, 8], F32)
            nc.sync.dma_start(avec[:], avec_d[:])
            umask = cpool.tile([32, JD * W], BF16)
            nc.sync.dma_start(umask[:], umask_d[:])
            mneg = cpool.tile([G, JD * W], BF16)
            nc.sync.dma_start(mneg[:], mneg_d[:])
            invc = cpool.tile([G, JD], F32)
            nc.sync.dma_start(invc[:], invc_d[:])
            wsp = cpool.tile([128, 4], F32)
            nc.sync.dma_start(wsp[:], wsp_d[:])
            mlpw = cpool.tile([G, 24], F32)
            nc.sync.dma_start(mlpw[:], mlp_d[:])

            def _one_iter():
              acc24 = epi.tile([G, 24], F32, tag="acc24")    # per-(chunk,s4) sums
              mx6 = epi.tile([G, 8], F32, tag="mx6")         # per-chunk maxes
              S1 = epi.tile([128, G * W], BF16, tag="S1")    # spatial rows 0-127
              S2 = epi.tile([64, G * W], BF16, tag="S2")     # spatial rows 128-191

              for ch in range(_NCH_ENV):
                  j, hh = divmod(ch, 2)
                  base = hh * CCOLS                    # column base in (h,w) flat

                  dt2 = dt2p.tile([128, CCOLS], BF16)
                  dtu2 = dtu2p.tile([128, CCOLS], BF16)
                  bc = bcp.tile([32, CCOLS], BF16)

                  for s4 in range(4):
                      cs = base + s4 * 1024
                      sl = slice(s4 * 1024, s4 * 1024 + 1024)
                      ft = ftp.tile([C, 2064], F32R, tag="ft")
                      nc.sync.dma_start(
                          ft[:].rearrange("p (h c) -> p h c", h=2),
                          feat_d[:].rearrange("p (h c) -> p h c", h=2)
                          [:, :, PAD + cs - 8: PAD + cs + 1024])
                      ftL = ft[:, 8:1032]
                      ftR = ft[:, 1032 + 8 - j: 2064 - j]

                      # dt stream
                      if 'd' not in _STREAMS: continue
                      pd = pproj.tile([128, 1024], F32, tag="proj")
                      for hv in range(2):
                          cv = slice(512 * hv, 512 * hv + 512)
                          nc.tensor.matmul(pd[:, cv], lhsT=wseL[:, 256:384],
                                           rhs=ftL[:, cv], start=True, stop=False)
                          nc.tensor.matmul(pd[:, cv], lhsT=wseR[:, 384:512],
                                           rhs=ftR[:, cv], start=False, stop=True)
                      dm = dtmpp.tile([128, 1024], BF16)
                      nc.scalar.activation(dm[:], pd[:], AF.Exp,
                                           bias=avec[:, 0:1], scale=1.0)
                      nc.scalar.activation(dt2[:, sl], dm[:], AF.Ln, bias=1.0,
                                           scale=1.0)

                      # u stream -> dtu = dt * u (PSUM operand TT)
                      if 'u' not in _STREAMS: continue
                      pu = pproj.tile([128, 1024], F32, tag="proj")
                      for hv in range(2):
                          cv = slice(512 * hv, 512 * hv + 512)
                          nc.tensor.matmul(pu[:, cv], lhsT=wseL[:, 0:128],
                                           rhs=ftL[:, cv], start=True, stop=False)
                          nc.tensor.matmul(pu[:, cv], lhsT=wseR[:, 128:256],
                                           rhs=ftR[:, cv], start=False, stop=True)
                      nc.vector.tensor_tensor(dtu2[:, sl], dt2[:, sl], pu[:],
                                              OP.mult)

                      # B|C|D stream, masked at eviction
                      if 'b' not in _STREAMS: continue
                      pb = pproj.tile([128, 1024], F32, tag="proj")
                      for hv in range(2):
                          cv = slice(512 * hv, 512 * hv + 512)
                          nc.tensor.matmul(pb[0:32, cv], lhsT=wseL[:, 512:544],
                                           rhs=ftL[:, cv], start=True, stop=False)
                          nc.tensor.matmul(pb[0:32, cv], lhsT=wseR[:, 544:576],
                                           rhs=ftR[:, cv], start=False, stop=True)
                      mview = umask[:, j * W:(j + 1) * W].unsqueeze(1) \
                          .broadcast_to((32, 8, W))
                      nc.vector.scalar_tensor_tensor(
                          bc[:, sl].rearrange("p (a b) -> p a b", b=W),
                          pb[0:32, :].rearrange("p (a b) -> p a b", b=W), 1.0,
                          mview, OP.mult, OP.mult)

                  # pair loop: a, b, scan
                  htiles = []
                  for p in (range(0) if _SKIP_PAIRS else range(4)):
                      bb = bbp.tile([128, CCOLS], BF16)
                      nc.scalar.dma_start(
                          bb[:],
                          bc[8 + 2 * p:8 + 2 * p + 2, :].unsqueeze(1)
                          .broadcast_to((2, 64, CCOLS)))

                      av = apl.tile([128, CCOLS], BF16)
                      nc.scalar.activation(av[:], dt2[:], AF.Exp,
                                           bias=0.0, scale=avec[:, 1 + p: 2 + p])
                      nc.vector.memset(
                          av[:].rearrange("p (h w) -> p h w", w=W)[:, :, 0:1], 0)

                      bt = bpl.tile([128, CCOLS], BF16)
                      nc.gpsimd.tensor_tensor(bt[:], dtu2[:], bb[:], OP.mult)

                      hT = hpl.tile([128, CCOLS], BF16)
                      nc.vector.tensor_tensor_scan(hT[:], av[:], bt[:], 0.0,
                                                   OP.mult, OP.add)
                      htiles.append(hT)

                  # csm: C rows broadcast to z layout [32p + q]
                  if not _SKIP_PAIRS:
                      csm = csmp.tile([128, CCOLS], BF16)
                      nc.scalar.dma_start(
                          csm[:],
                          bc[16:24, :].unsqueeze(1)
                          .broadcast_to((8, 16, CCOLS)))

                      tt = tpl.tile([128, CCOLS], BF16)
                      for s8 in range(4):
                          sl10 = slice(s8 * 1024, s8 * 1024 + 1024)
                          zp = pz.tile([128, 1024], F32, tag="zp")
                          for half in range(2):
                              zv = slice(512 * half, 512 * half + 512)
                              sl5 = slice(s8 * 1024 + 512 * half,
                                          s8 * 1024 + 512 * half + 512)
                              for p in range(4):
                                  nc.tensor.matmul(zp[32 * p:32 * p + 32, zv],
                                                   lhsT=wbf[:, 0:32],
                                                   rhs=htiles[p][:, sl5],
                                                   start=True, stop=True,
                                                   tile_position=(0, 32 * p))
                          nc.vector.scalar_tensor_tensor(tt[:, sl10], zp[:], 1.0,
                                                         csm[:, sl10], OP.mult, OP.mult)

                      cstg = cstgp.tile([8, CCOLS], BF16)
                      for s4 in range(4):
                          sl = slice(s4 * 1024, s4 * 1024 + 1024)
                          cp = pc.tile([8, 1024], F32, tag="cp")
                          for hv in range(2):
                              cv = slice(512 * hv, 512 * hv + 512)
                              cg = slice(s4 * 1024 + 512 * hv, s4 * 1024 + 512 * hv + 512)
                              nc.tensor.matmul(cp[:, cv], lhsT=wbf[:, 32:40],
                                               rhs=tt[:, cg], start=True, stop=False)
                              nc.tensor.matmul(cp[:, cv], lhsT=wbf[0:8, 40:48],
                                               rhs=bc[0:8, cg], start=False, stop=True)
                          nc.scalar.activation(
                              cstg[:, sl], cp[:], AF.Copy, bias=0.0, scale=1.0,
                              accum_out=acc24[:, ch * 4 + s4: ch * 4 + s4 + 1])

                      # masked per-chunk max for the channel-attn max pool
                      mscr = epi.tile([G, CCOLS], BF16, tag="mscr")
                      mview2 = mneg[:, j * W:(j + 1) * W].unsqueeze(1) \
                          .broadcast_to((G, HH, W))
                      nc.vector.tensor_tensor(
                          mscr[:].rearrange("p (h w) -> p h w", w=W),
                          cstg[:].rearrange("p (h w) -> p h w", w=W),
                          mview2, OP.add)
                      nc.vector.tensor_reduce(mx6[:, ch:ch + 1],
                                              mscr[:].unsqueeze(1), AX.X, OP.max)
                      # S: row = j*64 + hh*32 + h -> S1 rows 0..127, S2 rows 0..63
                      row0 = j * 64 + hh * 32
                      st_t, st_r = (S1, row0) if row0 < 128 else (S2, row0 - 128)
                      for g in range(G):
                          nc.scalar.dma_start(
                              st_t[st_r:st_r + 32, g * W:(g + 1) * W],
                              cstg[g:g + 1, :].rearrange("p (h w) -> p h w", w=W))

              if not _SKIP_EPI:
                  # ---------- epilogue ----------
                  avg = epi.tile([G, JD], F32)
                  nc.vector.tensor_reduce(
                      avg[:], acc24[:].rearrange("p (j r) -> p j r", r=8), AX.X, OP.add)
                  nc.vector.tensor_tensor(avg[:], avg[:], invc[:], OP.mult)
                  mx = epi.tile([G, JD], F32)
                  nc.vector.tensor_reduce(
                      mx[:], mx6[:, 0:6].rearrange("p (j r) -> p j r", r=2), AX.X, OP.max)

                  ppool = epi.tile([G, 6], F32)
                  nc.vector.tensor_copy(ppool[:, 0:3], avg[:])
                  nc.vector.tensor_copy(ppool[:, 3:6], mx[:])

                  z1p = pc.tile([4, 6], F32, tag="cp")
                  nc.tensor.matmul(z1p[:], lhsT=mlpw[:, 0:4], rhs=ppool[:],
                                   start=True, stop=True)
                  z1 = epi.tile([5, 6], F32)
                  nc.scalar.activation(z1[0:4, :], z1p[:], AF.Relu,
                                       bias=mlpw[0:4, 12:13], scale=1.0)
                  nc.sync.dma_start(z1[4:5, 0:6], mlpw[0:1, 16:22])
                  gp = pc.tile([3, 8], F32, tag="cp")
                  nc.tensor.matmul(gp[:], lhsT=z1[:, 0:3], rhs=mlpw[0:5, 4:12],
                                   start=True, stop=False)
                  nc.tensor.matmul(gp[:], lhsT=z1[:, 3:6], rhs=mlpw[0:5, 4:12],
                                   start=False, stop=True)
                  chg = epi.tile([3, 8], F32)
                  nc.scalar.activation(chg[:], gp[:], AF.Sigmoid, bias=0.0, scale=1.0)

                  gb1 = epi.tile([128, 8], F32)
                  gb2 = epi.tile([64, 8], F32)
                  for jj in range(2):
                      nc.sync.dma_start(
                          gb1[64 * jj:64 * jj + 64, :],
                          chg[jj:jj + 1, :].unsqueeze(1).broadcast_to((1, 64, 8)))
                  nc.sync.dma_start(
                      gb2[:], chg[2:3, :].unsqueeze(1).broadcast_to((1, 64, 8)))

                  for (Sg, gb, rows, obase) in ((S1, gb1, 128, 0), (S2, gb2, 64, 128)):
                      Sgf = epi.tile([rows, G * W], BF16, tag="sgf")
                      gview = gb[0:rows, :].unsqueeze(2).broadcast_to((rows, G, W))
                      nc.vector.tensor_tensor(
                          Sgf[:].rearrange("p (a b) -> p a b", b=W),
                          Sg[:].rearrange("p (a b) -> p a b", b=W), gview, OP.mult)
                      sv = Sgf[:].rearrange("p (g w) -> p w g", g=G)
                      ssum = epi.tile([rows, W], F32, tag="ss")
                      nc.vector.tensor_reduce(ssum[:], sv, AX.X, OP.add)
                      smx = epi.tile([rows, W], F32, tag="sm")
                      nc.vector.tensor_reduce(smx[:], sv, AX.X, OP.max)
                      q1 = epi.tile([rows, W], F32, tag="q1")
                      nc.vector.tensor_scalar_mul(q1[:], smx[:], wsp[0:rows, 1:2])
                      gi = epi.tile([rows, W], F32, tag="gi")
                      nc.vector.scalar_tensor_tensor(gi[:], ssum[:], wsp[0:rows, 0:1],
                                                     q1[:], OP.mult, OP.add)
                      sg = epi.tile([rows, W], F32, tag="sgate")
                      nc.scalar.activation(sg[:], gi[:], AF.Sigmoid,
                                           bias=wsp[0:rows, 2:3], scale=1.0)
                      O = epi.tile([rows, G * W], F32, tag="scr4k")
                      oview = sg[:].unsqueeze(1).broadcast_to((rows, G, W))
                      nc.vector.tensor_tensor(
                          O[:].rearrange("p (a b) -> p a b", b=W),
                          Sgf[:].rearrange("p (a b) -> p a b", b=W), oview, OP.mult)
                      nc.sync.dma_start(
                          out_d[obase:obase + rows].rearrange("r g w -> r (g w)"), O[:])
            for _it in range(_ITERS):
                _one_iter()

    nc.compile()
    return nc


def _host_inputs(inputs):
    """Build the 8 per-core input maps from the full problem inputs."""
    L = _f32(inputs["featuresL"])[0]          # [C,H,W]
    R = _f32(inputs["featuresR"])[0]
    W_in = _f32(inputs["W_in"])
    W_dt = _f32(inputs["W_dt"])
    b_dt = _f32(inputs["b_dt"])
    W_B = _f32(inputs["W_B"])
    W_C = _f32(inputs["W_C"])
    A = -np.exp(_f32(inputs["A_log"]))        # [E,S]
    D_skip = _f32(inputs["D_skip"])
    W_out = _f32(inputs["W_out"])
    W1, b1 = _f32(inputs["W1"]), _f32(inputs["b1"])
    W2, b2 = _f32(inputs["W2"]), _f32(inputs["b2"])
    w_sp, b_sp = _f32(inputs["w_sp"]), _f32(inputs["b_sp"])

    # stationary f32 weights [32, 576]
    idx = np.arange(128) % 64
    wse = np.zeros((2 * C, 576), np.float32)
    wse[0:32, 0:128] = W_in[0::2][:, idx]
    wse[32:64, 128:256] = W_in[1::2][:, idx]
    wse[0:32, 256:384] = W_dt[0::2][:, idx]
    wse[32:64, 384:512] = W_dt[1::2][:, idx]
    W_comb = W_in @ (D_skip[:, None] * W_out)        # [64(c), G]
    wse[0:32, 512:520] = W_comb[0::2]
    wse[0:32, 520:528] = W_B[0::2]
    wse[0:32, 528:536] = W_C[0::2]
    wse[32:64, 544:552] = W_comb[1::2]
    wse[32:64, 552:560] = W_B[1::2]
    wse[32:64, 560:568] = W_C[1::2]

    # bf16 stationaries [128, 48]
    wbf = np.zeros((128, 48), np.float32)
    for row in range(128):
        cc, e = divmod(row, 64)
        for q in range(32):
            c2, g = q // 16, q % 16
            if g < 8 and cc == c2:
                wbf[row, q] = W_out[e, g]
    for p4 in range(4):
        for local in range(32):
            c2, g = local // 16, local % 16
            if g < 8:
                wbf[32 * p4 + local, 32 + g] = 1.0
    wbf[0:8, 40:48] = np.eye(8, dtype=np.float32)

    avec = np.zeros((128, 8), np.float32)
    avec[:, 0] = b_dt[idx]
    for p4 in range(4):
        cc = np.arange(128) // 64
        avec[:, 1 + p4] = A[idx, 2 * p4 + cc]

    wspv = np.zeros((128, 4), np.float32)
    wspv[:, 0] = w_sp[0] / G
    wspv[:, 1] = w_sp[1]
    wspv[:, 2] = np.float32(np.asarray(b_sp).reshape(-1)[0]) if np.asarray(b_sp).size else 0.0

    mlpv = np.zeros((G, 24), np.float32)
    mlpv[:, 0:4] = W1
    mlpv[0:4, 4:12] = W2
    mlpv[4, 4:12] = 2.0 * b2
    mlpv[0:4, 12] = b1
    mlpv[0, 16:19] = 1.0

    maps = []
    wi = np.arange(W)
    for k in range(NCORES):
        d0 = JD * k
        Rsh = np.zeros_like(R)
        if d0 > 0:
            Rsh[:, :, d0:] = R[:, :, :-d0]
        else:
            Rsh = R
        feat = np.zeros((C, 2 * (PAD + HW)), np.float32)
        feat[:, PAD:PAD + HW] = L.reshape(C, HW)
        feat[:, 2 * PAD + HW:] = Rsh.reshape(C, HW)

        umask = np.zeros((32, JD * W), np.float32)
        for j in range(JD):
            umask[:, j * W:(j + 1) * W] = (wi >= d0 + j).astype(np.float32)[None]

        mneg = np.zeros((G, JD * W), np.float32)
        for j in range(JD):
            mneg[:, j * W:(j + 1) * W] = np.where(wi >= d0 + j, 0.0, -1e30)[None]

        invc = np.zeros((G, JD), np.float32)
        for j in range(JD):
            invc[:, j] = 1.0 / (H * (W - (d0 + j)))

        import ml_dtypes
        maps.append({
            "feat": feat,
            "wse": wse,
            "wbf": wbf.astype(ml_dtypes.bfloat16),
            "avec": avec,
            "umask": umask.astype(ml_dtypes.bfloat16),
            "mneg": mneg.astype(ml_dtypes.bfloat16),
            "invc": invc,
            "wsp": wspv,
            "mlp": mlpv,
        })
    return maps


def kernel(**inputs):
    from concourse.bass_utils import run_bass_kernel_spmd

    if "nc" not in _compiled:
        _compiled["nc"] = _build_program()
    nc = _compiled["nc"]

    maps = _host_inputs(inputs)
    res = run_bass_kernel_spmd(nc, maps, list(range(NCORES))).results

    vol = np.zeros((1, G, DV, H, W), np.float32)
    for k in range(NCORES):
        o = res[k]["out"].reshape(JD, H, G, W)        # [j,h,g,w]
        vol[0, :, JD * k:JD * k + JD] = np.transpose(o, (2, 0, 1, 3))
    return vol



# revision 14
# speedup vs baseline: 1.0218x; 1.0218x over previous
"""Trainium2 Bass kernel for nn_BuildCostVolume (stereo cost volume + Mamba scan).

Sharding: disparity axis (24) split as 3 per core across 8 cores.

Per-core algorithm (core k handles disparities d = 3k+j, j in 0..2):
  - Host pre-shifts featuresR right by 3k (zero-filled); in-kernel access
    patterns add the per-j shift (j in {0,1,2} is compile-time, SPMD-safe).
  - Projections u/dt/B/C/Dterm are computed from L and shifted-R features with
    even/odd split weights (channel interleave trick), on PE in float32r.
  - dt = softplus via Exp + Ln(x+1) on ACT (no softplus table on trn2).
  - Decay a = exp(A[e,s] * dt) via ACT per-partition scale, in an
    (s-pair x e) = 128-partition layout.
  - Mamba recurrence h = a*h + b via tensor_tensor_scan over flattened
    (row, w) with a[w=0]=0 so each image row restarts the scan.  The b-term
    multiplies (bt = dtu * B) run mostly on GPSIMD (Pool) to unload DVE.
  - PSUM evictions are partition-STACKED: the four per-s4 [32,1024] B/C/D
    blocks land in one [128,1024] PSUM tile and evict with a single masked
    stt; likewise the [8,*] cp blocks stack to [32,*].  Small SBUF->SBUF
    DMAs reshuffle back to the flat layouts consumers expect.
  - Channel attention (masked avg/max pool + MLP) and spatial attention
    computed on small repacked layouts; output written [j,h,g,w] and
    transposed on host.
"""
import os
import numpy as np

C, H, W, DV = 32, 64, 128, 24
_NCH_ENV = int(os.environ.get("KERNEL_NCH", "6"))
_SKIP_EPI = bool(int(os.environ.get("KERNEL_SKIP_EPI", "0")))
_ITERS = int(os.environ.get("KERNEL_ITERS", "1"))
E, S, G = 64, 8, 8
NCORES, JD = 8, 3          # cores, disparities per core
PAD = 8                    # leading zero columns in feature tensors
HH = 32                    # h rows per chunk
CCOLS = HH * W             # 4096 columns per chunk
HW = H * W                 # 8192

# (chunk, pair) combos whose bt multiply stays on DVE (rest go to Pool)
_BT_DVE = {(0, 0), (3, 0)}

_compiled = {}


def _f32(x):
    return np.ascontiguousarray(np.asarray(x, np.float32))


def _build_program():
    import concourse.bacc as bacc
    import concourse.mybir as mybir
    from concourse.tile import TileContext

    F32 = mybir.dt.float32
    F32R = mybir.dt.float32r
    BF16 = mybir.dt.bfloat16
    AF = mybir.ActivationFunctionType
    AX = mybir.AxisListType
    OP = mybir.AluOpType

    nc = bacc.Bacc("TRN2", target_bir_lowering=False, debug=False,
                   num_devices=NCORES)

    feat_d = nc.dram_tensor("feat", [C, 2 * (PAD + HW)], BF16, kind="ExternalInput").ap()
    wse_d = nc.dram_tensor("wse", [128, 768], BF16, kind="ExternalInput").ap()
    wbf_d = nc.dram_tensor("wbf", [128, 64], BF16, kind="ExternalInput").ap()
    avec_d = nc.dram_tensor("avec", [128, 8], F32, kind="ExternalInput").ap()
    umask_d = nc.dram_tensor("umask", [128, JD * W], BF16, kind="ExternalInput").ap()
    mneg_d = nc.dram_tensor("mneg", [128, JD * W], BF16, kind="ExternalInput").ap()
    invc_d = nc.dram_tensor("invc", [G, JD], F32, kind="ExternalInput").ap()
    wsp_d = nc.dram_tensor("wsp", [128, 4], F32, kind="ExternalInput").ap()
    mlp_d = nc.dram_tensor("mlp", [G, 24], F32, kind="ExternalInput").ap()
    out_d = nc.dram_tensor("out", [JD * H, G, W], F32, kind="ExternalOutput").ap()

    from contextlib import ExitStack
    with TileContext(nc) as tc:
        with ExitStack() as stack:
            _pool = lambda **kw: stack.enter_context(tc.tile_pool(**kw))
            cpool = _pool(name="const", bufs=1)
            ftp = _pool(name="ftp", bufs=2)
            dtmpp = _pool(name="dtmp", bufs=1)
            ubp = _pool(name="ubp", bufs=2)
            dt2p = _pool(name="dt2", bufs=2)
            dtu2p = _pool(name="dtu2", bufs=2)
            bcsp = _pool(name="bcs", bufs=1)
            bcp = _pool(name="bc", bufs=1)
            csmp = _pool(name="csm", bufs=1)
            apl = _pool(name="apool", bufs=2)
            bpl = _pool(name="bpool", bufs=2)
            hpl = _pool(name="hpool", bufs=4)
            tpl = _pool(name="tpool", bufs=1)
            cstg32p = _pool(name="cstg32p", bufs=1)
            cstgp = _pool(name="cstg", bufs=1)
            epi = _pool(name="epi", bufs=1)
            pproj = _pool(name="pproj", bufs=2, space="PSUM")
            pballp = _pool(name="pball", bufs=1, space="PSUM")
            pz = _pool(name="pz", bufs=1, space="PSUM")
            pc = _pool(name="pc", bufs=1, space="PSUM")

            _ld = mybir.InstLoadActFuncSet(
                name=nc.get_next_instruction_name(), act_func_set_id=6,
                ins=[], outs=[])
            nc.scalar.add_instruction(_ld)
            wse = cpool.tile([128, 768], BF16)
            nc.sync.dma_start(wse[:], wse_d[:])
            wbf = cpool.tile([128, 64], BF16)
            nc.sync.dma_start(wbf[:], wbf_d[:])
            avec = cpool.tile([128, 8], F32)
            nc.sync.dma_start(avec[:], avec_d[:])
            umask = cpool.tile([128, JD * W], BF16)
            nc.sync.dma_start(umask[:], umask_d[:])
            mneg = cpool.tile([128, JD * W], BF16)
            nc.sync.dma_start(mneg[:], mneg_d[:])
            invc = cpool.tile([G, JD], F32)
            nc.sync.dma_start(invc[:], invc_d[:])
            wsp = cpool.tile([128, 4], F32)
            nc.sync.dma_start(wsp[:], wsp_d[:])
            mlpw = cpool.tile([G, 24], F32)
            nc.sync.dma_start(mlpw[:], mlp_d[:])

            def _one_iter():
              acc128 = epi.tile([128, 12], F32, tag="acc128")  # per-(chunk,half) sums
              mx128 = epi.tile([128, 8], F32, tag="mx128")     # per-chunk maxes (stacked)
              S1 = epi.tile([128, G * W], BF16, tag="S1")    # spatial rows 0-127
              S2 = epi.tile([64, G * W], BF16, tag="S2")     # spatial rows 128-191

              for ch in range(_NCH_ENV):
                  j, hh = divmod(ch, 2)
                  base = hh * CCOLS                    # column base in (h,w) flat

                  dt2 = dt2p.tile([128, CCOLS], BF16)
                  dtu2 = dtu2p.tile([128, CCOLS], BF16)
                  pball = pballp.tile([128, 1024], F32, tag="pball")

                  ftall = ftp.tile([128, 2064], BF16, tag="ft")
                  for s4 in range(4):
                      cs = base + s4 * 1024
                      nc.sync.dma_start(
                          ftall[32 * s4:32 * s4 + 32, :]
                          .rearrange("p (h c) -> p h c", h=2),
                          feat_d[:].rearrange("p (h c) -> p h c", h=2)
                          [:, :, PAD + cs - 8: PAD + cs + 1024])

                  for s4 in range(4):
                      sl = slice(s4 * 1024, s4 * 1024 + 1024)
                      rs = slice(32 * s4, 32 * s4 + 32)
                      ftL = ftall[rs, 8:1032]
                      ftR = ftall[rs, 1032 + 8 - j: 2064 - j]

                      # dt stream (row-tiled: ifmap+weights at partitions 32*s4)
                      pd = pproj.tile([128, 1024], F32, tag="proj")
                      for hv in range(2):
                          cv = slice(512 * hv, 512 * hv + 512)
                          nc.tensor.matmul(pd[:, cv], lhsT=wse[rs, 256:384],
                                           rhs=ftL[:, cv], start=True, stop=False,
                                           tile_position=(32 * s4, 0))
                          nc.tensor.matmul(pd[:, cv], lhsT=wse[rs, 384:512],
                                           rhs=ftR[:, cv], start=False, stop=True,
                                           tile_position=(32 * s4, 0))
                      dm = dtmpp.tile([128, 1024], BF16)
                      nc.scalar.activation(dm[:], pd[:], AF.Exp,
                                           bias=avec[:, 0:1], scale=1.0)
                      nc.scalar.activation(dt2[:, sl], dm[:], AF.Ln, bias=1.0,
                                           scale=1.0)

                      # u stream: evict via ACT to bf16, dtu = dt * u on DVE 2x
                      pu = pproj.tile([128, 1024], F32, tag="proj")
                      for hv in range(2):
                          cv = slice(512 * hv, 512 * hv + 512)
                          nc.tensor.matmul(pu[:, cv], lhsT=wse[rs, 0:128],
                                           rhs=ftL[:, cv], start=True, stop=False,
                                           tile_position=(32 * s4, 0))
                          nc.tensor.matmul(pu[:, cv], lhsT=wse[rs, 128:256],
                                           rhs=ftR[:, cv], start=False, stop=True,
                                           tile_position=(32 * s4, 0))
                      ub = ubp.tile([128, 1024], BF16, tag="ub")
                      nc.scalar.activation(ub[:], pu[:], AF.Copy, bias=0.0,
                                           scale=1.0)
                      nc.vector.tensor_tensor(dtu2[:, sl], dt2[:, sl], ub[:],
                                              OP.mult)

                      # B|C|D stream: block-diag weights, full-128 contraction,
                      # bf16 tiles legally write the 32-row psum quadrant
                      fL = ftall[:, 8:1032]
                      fR = ftall[:, 1032 + 8 - j: 2064 - j]
                      for hv in range(2):
                          cv = slice(512 * hv, 512 * hv + 512)
                          nc.tensor.matmul(pball[rs, 512 * hv:512 * hv + 512],
                                           lhsT=wse[:, 512 + 32 * s4:512 + 32 * s4 + 32],
                                           rhs=fL[:, cv], start=True, stop=False,
                                           tile_position=(0, 32 * s4))
                          nc.tensor.matmul(pball[rs, 512 * hv:512 * hv + 512],
                                           lhsT=wse[:, 640 + 32 * s4:640 + 32 * s4 + 32],
                                           rhs=fR[:, cv], start=False, stop=True,
                                           tile_position=(0, 32 * s4))

                  # one masked eviction for all four s4 blocks, then reshuffle
                  bcs = bcsp.tile([128, 1024], BF16)
                  mview = umask[:, j * W:(j + 1) * W].unsqueeze(1) \
                      .broadcast_to((128, 8, W))
                  nc.vector.scalar_tensor_tensor(
                      bcs[:].rearrange("p (a b) -> p a b", b=W),
                      pball[:].rearrange("p (a b) -> p a b", b=W), 1.0,
                      mview, OP.mult, OP.mult)
                  bc = bcp.tile([32, CCOLS], BF16)
                  for s4 in range(4):
                      nc.sync.dma_start(bc[:, s4 * 1024:(s4 + 1) * 1024],
                                        bcs[32 * s4:32 * s4 + 32, :])

                  # pair loop: a, b, scan
                  htiles = []
                  for p in range(4):
                      bb = bpl.tile([128, CCOLS], BF16, tag="bb")
                      nc.scalar.dma_start(
                          bb[:],
                          bc[8 + 2 * p:8 + 2 * p + 2, :].unsqueeze(1)
                          .broadcast_to((2, 64, CCOLS)))

                      av = apl.tile([128, CCOLS], BF16)
                      nc.scalar.activation(av[:], dt2[:], AF.Exp,
                                           bias=0.0, scale=avec[:, 1 + p: 2 + p])
                      nc.vector.memset(
                          av[:].rearrange("p (h w) -> p h w", w=W)[:, :, 0:1], 0)

                      bt = bpl.tile([128, CCOLS], BF16, tag="bt")
                      eng = nc.vector if (ch, p) in _BT_DVE else nc.gpsimd
                      eng.tensor_tensor(bt[:], dtu2[:], bb[:], OP.mult)

                      hT = hpl.tile([128, CCOLS], BF16)
                      nc.vector.tensor_tensor_scan(hT[:], av[:], bt[:], 0.0,
                                                   OP.mult, OP.add)
                      htiles.append(hT)

                  # csm: C rows broadcast to z layout [32p + q]
                  csm = csmp.tile([128, CCOLS], BF16)
                  nc.scalar.dma_start(
                      csm[:],
                      bc[16:24, :].unsqueeze(1)
                      .broadcast_to((8, 16, CCOLS)))

                  tt = tpl.tile([128, CCOLS], BF16)
                  for s8 in range(8):
                      sl5 = slice(s8 * 512, s8 * 512 + 512)
                      zp = pz.tile([128, 512], F32, tag="zp")
                      for p in range(4):
                          nc.tensor.matmul(zp[32 * p:32 * p + 32, :],
                                           lhsT=wbf[:, 0:32],
                                           rhs=htiles[p][:, sl5],
                                           start=True, stop=True,
                                           tile_position=(0, 32 * p))
                      nc.vector.scalar_tensor_tensor(tt[:, sl5], zp[:], 1.0,
                                                     csm[:, sl5], OP.mult, OP.mult)
                  # D-term rides unused tt partitions 8:16 into the cp matmul
                  nc.sync.dma_start(tt[8:16, :], bc[0:8, :])

                  # cp blocks stacked at 32-row offsets; evict once per half
                  cstg32 = cstg32p.tile([128, 1024], BF16)
                  for half in range(2):
                      cpall = pc.tile([128, 512], F32, tag="cp")
                      for s4 in range(4):
                          cg = slice(s4 * 1024 + 512 * half,
                                     s4 * 1024 + 512 * half + 512)
                          nc.tensor.matmul(cpall[32 * s4:32 * s4 + 32, :],
                                           lhsT=wbf[:, 32:64],
                                           rhs=tt[:, cg], start=True, stop=True,
                                           tile_position=(0, 32 * s4))
                      nc.scalar.activation(
                          cstg32[:, 512 * half:512 * half + 512], cpall[:],
                          AF.Copy, bias=0.0, scale=1.0,
                          accum_out=acc128[:, ch * 2 + half: ch * 2 + half + 1])

                  # stacked masked max for the channel-attn max pool
                  mscr = epi.tile([128, 1024], BF16, tag="mscr")
                  mview2 = mneg[:, j * W:(j + 1) * W].unsqueeze(1) \
                      .broadcast_to((128, 8, W))
                  nc.vector.tensor_tensor(
                      mscr[:].rearrange("p (h w) -> p h w", w=W),
                      cstg32[:].rearrange("p (h w) -> p h w", w=W),
                      mview2, OP.add)
                  nc.vector.tensor_reduce(mx128[:, ch:ch + 1],
                                          mscr[:].unsqueeze(1), AX.X, OP.max)

                  # reshuffle to flat [8, CCOLS] for spatial staging
                  cstg = cstgp.tile([8, CCOLS], BF16)
                  for s4 in range(4):
                      nc.sync.dma_start(cstg[:, s4 * 1024:(s4 + 1) * 1024],
                                        cstg32[32 * s4:32 * s4 + 8, :])
                  # S: row = j*64 + hh*32 + h -> S1 rows 0..127, S2 rows 0..63
                  row0 = j * 64 + hh * 32
                  st_t, st_r = (S1, row0) if row0 < 128 else (S2, row0 - 128)
                  for g in range(G):
                      nc.scalar.dma_start(
                          st_t[st_r:st_r + 32, g * W:(g + 1) * W],
                          cstg[g:g + 1, :].rearrange("p (h w) -> p h w", w=W))

              if not _SKIP_EPI:
                  # ---------- epilogue ----------
                  # un-stack acc32/mx32: accT[g, (j hh half s4)], mxT[g, (j hh s4)]
                  accT = epi.tile([G, 48], F32, tag="accT")
                  mxT = epi.tile([G, 24], F32, tag="mxT")
                  for s4 in range(4):
                      nc.sync.dma_start(
                          accT[:].rearrange("p (c s) -> p c s", s=4)
                          [:, :, s4:s4 + 1],
                          acc128[32 * s4:32 * s4 + 8, :].unsqueeze(2))
                      nc.sync.dma_start(
                          mxT[:].rearrange("p (c s) -> p c s", s=4)
                          [:, :, s4:s4 + 1],
                          mx128[32 * s4:32 * s4 + 8, 0:6].unsqueeze(2))

                  avg = epi.tile([G, JD], F32)
                  nc.vector.tensor_reduce(
                      avg[:], accT[:].rearrange("p (j r) -> p j r", r=16),
                      AX.X, OP.add)
                  nc.vector.tensor_tensor(avg[:], avg[:], invc[:], OP.mult)
                  mx = epi.tile([G, JD], F32)
                  nc.vector.tensor_reduce(
                      mx[:], mxT[:].rearrange("p (j r) -> p j r", r=8),
                      AX.X, OP.max)

                  ppool = epi.tile([G, 6], F32)
                  nc.vector.tensor_copy(ppool[:, 0:3], avg[:])
                  nc.vector.tensor_copy(ppool[:, 3:6], mx[:])

                  z1p = pc.tile([4, 6], F32, tag="cp")
                  nc.tensor.matmul(z1p[:], lhsT=mlpw[:, 0:4], rhs=ppool[:],
                                   start=True, stop=True)
                  z1 = epi.tile([5, 6], F32)
                  nc.scalar.activation(z1[0:4, :], z1p[:], AF.Relu,
                                       bias=mlpw[0:4, 12:13], scale=1.0)
                  nc.sync.dma_start(z1[4:5, 0:6], mlpw[0:1, 16:22])
                  gp = pc.tile([3, 8], F32, tag="cp")
                  nc.tensor.matmul(gp[:], lhsT=z1[:, 0:3], rhs=mlpw[0:5, 4:12],
                                   start=True, stop=False)
                  nc.tensor.matmul(gp[:], lhsT=z1[:, 3:6], rhs=mlpw[0:5, 4:12],
                                   start=False, stop=True)
                  chg = epi.tile([3, 8], F32)
                  nc.scalar.activation(chg[:], gp[:], AF.Sigmoid, bias=0.0, scale=1.0)

                  gb1 = epi.tile([128, 8], F32)
                  gb2 = epi.tile([64, 8], F32)
                  for jj in range(2):
                      nc.sync.dma_start(
                          gb1[64 * jj:64 * jj + 64, :],
                          chg[jj:jj + 1, :].unsqueeze(1).broadcast_to((1, 64, 8)))
                  nc.sync.dma_start(
                      gb2[:], chg[2:3, :].unsqueeze(1).broadcast_to((1, 64, 8)))

                  for (Sg, gb, rows, obase) in ((S1, gb1, 128, 0), (S2, gb2, 64, 128)):
                      Sgf = epi.tile([rows, G * W], BF16, tag="sgf")
                      gview = gb[0:rows, :].unsqueeze(2).broadcast_to((rows, G, W))
                      nc.vector.tensor_tensor(
                          Sgf[:].rearrange("p (a b) -> p a b", b=W),
                          Sg[:].rearrange("p (a b) -> p a b", b=W), gview, OP.mult)
                      sv = Sgf[:].rearrange("p (g w) -> p w g", g=G)
                      ssum = epi.tile([rows, W], F32, tag="ss")
                      nc.vector.tensor_reduce(ssum[:], sv, AX.X, OP.add)
                      smx = epi.tile([rows, W], F32, tag="sm")
                      nc.vector.tensor_reduce(smx[:], sv, AX.X, OP.max)
                      q1 = epi.tile([rows, W], F32, tag="q1")
                      nc.vector.tensor_scalar_mul(q1[:], smx[:], wsp[0:rows, 1:2])
                      gi = epi.tile([rows, W], F32, tag="gi")
                      nc.vector.scalar_tensor_tensor(gi[:], ssum[:], wsp[0:rows, 0:1],
                                                     q1[:], OP.mult, OP.add)
                      sg = epi.tile([rows, W], F32, tag="sgate")
                      nc.scalar.activation(sg[:], gi[:], AF.Sigmoid,
                                           bias=wsp[0:rows, 2:3], scale=1.0)
                      O = epi.tile([rows, G * W], F32, tag="scr4k")
                      oview = sg[:].unsqueeze(1).broadcast_to((rows, G, W))
                      nc.vector.tensor_tensor(
                          O[:].rearrange("p (a b) -> p a b", b=W),
                          Sgf[:].rearrange("p (a b) -> p a b", b=W), oview, OP.mult)
                      nc.sync.dma_start(
                          out_d[obase:obase + rows].rearrange("r g w -> r (g w)"), O[:])
            for _it in range(_ITERS):
                _one_iter()

    nc.compile()
    return nc


def _host_inputs(inputs):
    """Build the 8 per-core input maps from the full problem inputs."""
    L = _f32(inputs["featuresL"])[0]          # [C,H,W]
    R = _f32(inputs["featuresR"])[0]
    W_in = _f32(inputs["W_in"])
    W_dt = _f32(inputs["W_dt"])
    b_dt = _f32(inputs["b_dt"])
    W_B = _f32(inputs["W_B"])
    W_C = _f32(inputs["W_C"])
    A = -np.exp(_f32(inputs["A_log"]))        # [E,S]
    D_skip = _f32(inputs["D_skip"])
    W_out = _f32(inputs["W_out"])
    W1, b1 = _f32(inputs["W1"]), _f32(inputs["b1"])
    W2, b2 = _f32(inputs["W2"]), _f32(inputs["b2"])
    w_sp, b_sp = _f32(inputs["w_sp"]), _f32(inputs["b_sp"])

    # stationary bf16 weights [128, 768]: 4x-replicated rows (per-s4 row
    # tiles) for u/dt, block-diagonal columns for the B|C|D stream
    idx = np.arange(128) % 64
    wse = np.zeros((128, 768), np.float32)
    W_comb = W_in @ (D_skip[:, None] * W_out)        # [64(c), G]
    bL = np.zeros((32, 32), np.float32)
    bR = np.zeros((32, 32), np.float32)
    bL[:, 0:8], bL[:, 8:16], bL[:, 16:24] = W_comb[0::2], W_B[0::2], W_C[0::2]
    bR[:, 0:8], bR[:, 8:16], bR[:, 16:24] = W_comb[1::2], W_B[1::2], W_C[1::2]
    for q in range(4):
        r = slice(32 * q, 32 * q + 32)
        wse[r, 0:128] = W_in[0::2][:, idx]
        wse[r, 128:256] = W_in[1::2][:, idx]
        wse[r, 256:384] = W_dt[0::2][:, idx]
        wse[r, 384:512] = W_dt[1::2][:, idx]
        wse[r, 512 + 32 * q:512 + 32 * q + 32] = bL
        wse[r, 640 + 32 * q:640 + 32 * q + 32] = bR

    # bf16 stationaries [128, 64]
    wbf = np.zeros((128, 64), np.float32)
    for row in range(128):
        cc, e = divmod(row, 64)
        for q in range(32):
            c2, g = q // 16, q % 16
            if g < 8 and cc == c2:
                wbf[row, q] = W_out[e, g]
    for p4 in range(4):
        for local in range(32):
            c2, g = local // 16, local % 16
            if g < 8:
                wbf[32 * p4 + local, 32 + g] = 1.0
    # D-term: unused z-layout partitions 8:16 carry bc rows 0:8 (see kernel)
    for r in range(8):
        wbf[8 + r, 32 + r] += 1.0

    avec = np.zeros((128, 8), np.float32)
    avec[:, 0] = b_dt[idx]
    for p4 in range(4):
        cc = np.arange(128) // 64
        avec[:, 1 + p4] = A[idx, 2 * p4 + cc]

    wspv = np.zeros((128, 4), np.float32)
    wspv[:, 0] = w_sp[0] / G
    wspv[:, 1] = w_sp[1]
    wspv[:, 2] = np.float32(np.asarray(b_sp).reshape(-1)[0]) if np.asarray(b_sp).size else 0.0

    mlpv = np.zeros((G, 24), np.float32)
    mlpv[:, 0:4] = W1
    mlpv[0:4, 4:12] = W2
    mlpv[4, 4:12] = 2.0 * b2
    mlpv[0:4, 12] = b1
    mlpv[0, 16:19] = 1.0

    maps = []
    wi = np.arange(W)
    for k in range(NCORES):
        d0 = JD * k
        Rsh = np.zeros_like(R)
        if d0 > 0:
            Rsh[:, :, d0:] = R[:, :, :-d0]
        else:
            Rsh = R
        feat = np.zeros((C, 2 * (PAD + HW)), np.float32)
        feat[:, PAD:PAD + HW] = L.reshape(C, HW)
        feat[:, 2 * PAD + HW:] = Rsh.reshape(C, HW)

        umask = np.zeros((128, JD * W), np.float32)
        for j in range(JD):
            umask[:, j * W:(j + 1) * W] = (wi >= d0 + j).astype(np.float32)[None]

        mneg = np.zeros((128, JD * W), np.float32)
        for j in range(JD):
            mneg[:, j * W:(j + 1) * W] = np.where(wi >= d0 + j, 0.0, -1e30)[None]

        invc = np.zeros((G, JD), np.float32)
        for j in range(JD):
            invc[:, j] = 1.0 / (H * (W - (d0 + j)))

        import ml_dtypes
        maps.append({
            "feat": feat.astype(ml_dtypes.bfloat16),
            "wse": wse.astype(ml_dtypes.bfloat16),
            "wbf": wbf.astype(ml_dtypes.bfloat16),
            "avec": avec,
            "umask": umask.astype(ml_dtypes.bfloat16),
            "mneg": mneg.astype(ml_dtypes.bfloat16),
            "invc": invc,
            "wsp": wspv,
            "mlp": mlpv,
        })
    return maps


def kernel(**inputs):
    from concourse.bass_utils import run_bass_kernel_spmd

    if "nc" not in _compiled:
        _compiled["nc"] = _build_program()
    nc = _compiled["nc"]

    maps = _host_inputs(inputs)
    res = run_bass_kernel_spmd(nc, maps, list(range(NCORES))).results

    vol = np.zeros((1, G, DV, H, W), np.float32)
    for k in range(NCORES):
        o = res[k]["out"].reshape(JD, H, G, W)        # [j,h,g,w]
        vol[0, :, JD * k:JD * k + JD] = np.transpose(o, (2, 0, 1, 3))
    return vol
